# revision 22
# baseline (speedup 1.0000x reference)
"""Trainium2 Bass kernel for a dense transformer block (B=8, T=1024, C=1024, H=16, FF=4096).

Sharding: data-parallel over batch — one batch element per NeuronCore (8 cores),
no collectives. Host does weight fake-quantization (exact, per reference formula),
LayerNorm gamma/beta folding into the adjacent projections, transposition to the
matmul-friendly [K, N] layouts, and bf16 casting. The device kernel computes, per
core, the whole block for its batch element:

  h1T = LN1(x)^T           (bf16, C on partitions; PE transposes)
  per head-pair p (software pipeline):
    qT/kT [d, t] matmuls -> S^T[s,t] = kT.T qT (K=64, two heads in disjoint PE
    row groups) -> exp (ACT, scale=C^-0.5) -> causal mask mult on diagonal tiles
    -> attV for pair p-1: attT_aug[65, t] = v_aug.T @ expT (PSUM, row 64 =
    softmax sums) -> recip(sums row) -> DMA round-trip broadcast of r ->
    attT *= r
  y = attT.T @ WpT; xnew = x + y; h2T = LN2(xnew)^T
  f1T[ff, t] = relu(W1T.T @ h2T); y2 = f1T.T @ W2T; out = xnew + y2

All matmuls are bf16 with fp32 PSUM accumulation. The weight quantization grid
(multiples of 2^e with tiny integer multipliers) is exactly representable in bf16.
"""

import os
import numpy as np
import ml_dtypes

B, T, C, H = 8, 1024, 1024, 16
HS = C // H          # 64
FF = 4 * C           # 4096
EPS = 1e-5
NT = T // 128        # 8 t-tiles
NCI = C // 128       # 8 c-tiles
NFF = FF // 128      # 32 ff-tiles
VW = 66              # per-head stride in v_aug (64 v cols + ones col + pad)
SM_SCALE = 1.0 / 32.0  # C ** -0.5

_CACHE = {}


# ----------------------------------------------------------------------------
# host-side math (exact reference semantics)
# ----------------------------------------------------------------------------

def _quant_weight(W, e, b):
    W = np.asarray(W, np.float32)
    e = np.asarray(e, np.float32)
    b = np.asarray(b, np.float32)
    b_rel = np.maximum(b, 0.0)
    mn = np.where(b_rel > 0, -(2.0 ** (b_rel - 1)), 0.0)
    mx = np.where(b_rel > 0, 2.0 ** (b_rel - 1) - 1.0, 0.0)
    qw = np.clip((2.0 ** (-e)) * W, mn, mx)
    w = np.round(qw)  # round-half-even, same as jnp.round
    return ((2.0 ** e) * w).astype(np.float32)


def _prep(inputs):
    f32 = np.float32
    bf16 = ml_dtypes.bfloat16
    g1 = np.asarray(inputs["g1"], f32)
    be1 = np.asarray(inputs["be1"], f32)
    g2 = np.asarray(inputs["g2"], f32)
    be2 = np.asarray(inputs["be2"], f32)

    Wq = _quant_weight(inputs["Wq"], inputs["eq"], inputs["bq"])  # [H,HS,C]
    Wk = _quant_weight(inputs["Wk"], inputs["ek"], inputs["bk"])
    Wv = _quant_weight(inputs["Wv"], inputs["ev"], inputs["bv"])
    Wp = _quant_weight(inputs["Wp"], inputs["ep"], inputs["bp"])  # [C,C]
    W1 = _quant_weight(inputs["W1"], inputs["e1"], inputs["b1"])  # [FF,C]
    W2 = _quant_weight(inputs["W2"], inputs["e2"], inputs["b2"])  # [C,FF]

    def qkvT(W):
        # [H,HS,C] -> fold g1 -> [C, H*HS]
        Wf = W * g1[None, None, :]
        return np.ascontiguousarray(Wf.reshape(H * HS, C).T).astype(bf16)

    def qkv_bias(W):
        # [H,HS,C] @ be1 -> [H*HS] -> [128, 8] with (r, p) = bias[128p + r]
        bias = (W.reshape(H * HS, C) @ be1).astype(f32)
        return np.ascontiguousarray(bias.reshape(8, 128).T)

    fp8 = ml_dtypes.float8_e4m3fn

    def qkv_pair(W):
        # [C, H*HS] -> [head-pair, 128(c-row), c-pair, 2, 128] fp8 DoubleRow
        # stationary layout; one DMA per head pair.
        WT = np.asarray(qkvT(W), np.float32)  # [C, H*HS]
        return np.ascontiguousarray(
            WT.reshape(4, 2, 128, 8, 128)
            .transpose(3, 2, 0, 1, 4).reshape(8, 128, NCI * 128)).astype(fp8)

    def wv_pair(W):
        # [C, H*HS] -> [c-pair, 128(c-row), 2, H*HS] fp8 DoubleRow moving
        WT = np.asarray(qkvT(W), np.float32)
        return np.ascontiguousarray(
            WT.reshape(4, 2, 128, C).transpose(0, 2, 1, 3)).astype(fp8)

    d = {
        "wqb": qkv_pair(Wq), "wkb": qkv_pair(Wk), "wvb": wv_pair(Wv),
        "qb": qkv_bias(Wq), "kb": qkv_bias(Wk),
        "wpb": np.ascontiguousarray(
            Wp.T.reshape(4, 2, 128, C).transpose(0, 2, 1, 3)).astype(fp8),
        # W1T [C, FF] -> [f, c_row(128), c-pair, 2, f_col(128)] fp8
        "w1b": np.ascontiguousarray(
            (W1 * g2[None, :]).T.reshape(4, 2, 128, NFF, 128)
            .transpose(3, 2, 0, 1, 4).reshape(NFF, 128, NCI * 128)).astype(fp8),
        # W2T [FF, C] -> [f-pair, 128(f-row), 2, C] fp8 DoubleRow moving
        "w2b": np.ascontiguousarray(
            W2.T.reshape(16, 2, 128, C).transpose(0, 2, 1, 3)).astype(fp8),
    }
    # b1eff[ff] = bias1 + W1 @ be2 ; laid out [128, 32] (partition r, col f)
    b1eff = (np.asarray(inputs["bias1"], f32) + W1 @ be2).astype(f32)
    d["b1e"] = np.ascontiguousarray(b1eff.reshape(NFF, 128).T)
    # v bias (from be1 through Wv), padded into the VW-stride layout
    vb = (Wv.reshape(H * HS, C) @ be1).astype(f32)                       # [H*HS]
    vb_pad = np.zeros(H * VW, f32)
    for h in range(H):
        vb_pad[h * VW: h * VW + HS] = vb[h * HS:(h + 1) * HS]
    d["vbpad"] = vb_pad
    d["biasp"] = np.asarray(inputs["biasp"], f32)
    d["bias2"] = np.asarray(inputs["bias2"], f32)
    # causal mask for diagonal blocks in S^T orientation: keep t_local >= s_local
    mask = (np.arange(128)[None, :] >= np.arange(128)[:, None])
    d["mask"] = mask.astype(bf16)
    qb = qkv_bias(Wq)
    kb = qkv_bias(Wk)
    flags = {
        "vb_nz": bool(np.any(vb != 0)),
        "qb_nz": bool(np.any(qb != 0)),
        "kb_nz": bool(np.any(kb != 0)),
        "biasp_nz": bool(np.any(d["biasp"] != 0)),
        "bias2_nz": bool(np.any(d["bias2"] != 0)),
    }
    return d, flags


# ----------------------------------------------------------------------------
# device kernel
# ----------------------------------------------------------------------------

def build(flags):
    import concourse.bass as bass
    import concourse.tile as tile
    from concourse import bacc, mybir

    f32 = mybir.dt.float32
    bf16 = mybir.dt.bfloat16
    AF = mybir.ActivationFunctionType
    OP = mybir.AluOpType

    nc = bacc.Bacc("TRN2", target_bir_lowering=False)

    xd = nc.dram_tensor("x", [T, C], f32, kind="ExternalInput")
    fp8 = mybir.dt.float8e4
    DR = mybir.MatmulPerfMode.DoubleRow
    wqb = nc.dram_tensor("wqb", [8, 128, C], fp8, kind="ExternalInput")
    wkb = nc.dram_tensor("wkb", [8, 128, C], fp8, kind="ExternalInput")
    wvb = nc.dram_tensor("wvb", [4, 128, 2, C], fp8, kind="ExternalInput")
    qbd = nc.dram_tensor("qb", [128, 8], f32, kind="ExternalInput")
    kbd = nc.dram_tensor("kb", [128, 8], f32, kind="ExternalInput")
    wpb = nc.dram_tensor("wpb", [4, 128, 2, C], fp8, kind="ExternalInput")
    w1b = nc.dram_tensor("w1b", [NFF, 128, NCI * 128], fp8,
                         kind="ExternalInput")
    w2b = nc.dram_tensor("w2b", [16, 128, 2, C], fp8, kind="ExternalInput")
    b1ed = nc.dram_tensor("b1e", [128, NFF], f32, kind="ExternalInput")
    maskd = nc.dram_tensor("mask", [128, 128], bf16, kind="ExternalInput")
    vbpd = nc.dram_tensor("vbpad", [H * VW], f32, kind="ExternalInput")
    biaspd = nc.dram_tensor("biasp", [C], f32, kind="ExternalInput")
    bias2d = nc.dram_tensor("bias2", [C], f32, kind="ExternalInput")
    outd = nc.dram_tensor("out", [T, C], f32, kind="ExternalOutput")
    # softmax 1/sum rows round-trip scratch ("Internal" DRAM fails NEFF load
    # under axon/bass2jax, so expose as an ignored output)
    rscr = nc.dram_tensor("rscr", [8, 64, T], f32, kind="ExternalOutput")

    def bcast_dram_row(vec_ap, p, n):
        # DRAM [n] broadcast across p partitions -> AP [p, n]
        return bass.AP(tensor=vec_ap.tensor, offset=vec_ap.offset,
                       ap=[[0, p], [1, n]])

    with tile.TileContext(nc) as tc, \
         tc.tile_pool(name="consts", bufs=1) as consts, \
         tc.tile_pool(name="xpool", bufs=1) as xpool, \
         tc.tile_pool(name="hpool", bufs=1) as hpool, \
         tc.tile_pool(name="ln_tmp", bufs=3) as ln_tmp:

        # ---- constants ----
        from concourse.masks import make_identity
        ident = consts.tile([128, 128], bf16, name="ident")
        make_identity(nc, ident[:])
        qb_sb = consts.tile([128, 8], f32, name="qb_sb")
        kb_sb = consts.tile([128, 8], f32, name="kb_sb")
        b1e_sb = consts.tile([128, NFF], f32, name="b1e_sb")
        mask_sb = consts.tile([128, 128], bf16, name="mask_sb")
        eps_sb = consts.tile([128, 1], f32, name="eps_sb")
        nc.vector.memset(eps_sb[:], EPS)

        # ---- x tiles first (LN1 critical path), then the other consts ----
        x_sb = []
        for t in range(NT):
            xt = xpool.tile([128, C], f32, name=f"x{t}")
            nc.sync.dma_start(xt[:], xd[128 * t:128 * (t + 1), :])
            x_sb.append(xt)
        if flags["qb_nz"]:
            nc.sync.dma_start(qb_sb[:], qbd[:, :])
        if flags["kb_nz"]:
            nc.sync.dma_start(kb_sb[:], kbd[:, :])
        nc.sync.dma_start(b1e_sb[:], b1ed[:, :])
        nc.sync.dma_start(mask_sb[:], maskd[:, :])
        if flags["vb_nz"]:
            vb_sb = consts.tile([128, H * VW], f32, name="vb_sb")
            nc.sync.dma_start(vb_sb[:], bcast_dram_row(vbpd[:], 128, H * VW))
        if flags["biasp_nz"]:
            bp_sb = consts.tile([128, C], f32, name="bp_sb")
            nc.sync.dma_start(bp_sb[:], bcast_dram_row(biaspd[:], 128, C))
        if flags["bias2_nz"]:
            b2_sb = consts.tile([128, C], f32, name="b2_sb")
            nc.sync.dma_start(b2_sb[:], bcast_dram_row(bias2d[:], 128, C))

        hTall = hpool.tile([128, 4, 2, T], fp8, name="hTall")

        def layer_norm_to_hT(ps_tr):
            """LN over x tiles -> bf16 h tiles -> transpose into hT."""
            for t in range(NT):
                xt = x_sb[t]
                stats = ln_tmp.tile([128, 2, 6], f32, tag="lnstats")
                nc.vector.bn_stats(stats[:, 0, :], xt[:, 0:512])
                nc.vector.bn_stats(stats[:, 1, :], xt[:, 512:1024])
                mv = ln_tmp.tile([128, 2], f32, tag="lnmv")
                nc.vector.bn_aggr(mv[:], stats[:])
                rstd = ln_tmp.tile([128, 1], f32, tag="lnrstd")
                # rstd = 1 / sqrt(var + EPS)
                nc.scalar.activation(rstd[:], mv[:, 1:2], AF.Sqrt, bias=eps_sb[:])
                nc.vector.reciprocal(rstd[:], rstd[:])
                # nmr = -mean * rstd; h = x * rstd + nmr  (on ACT)
                nmr = ln_tmp.tile([128, 1], f32, tag="lnnmr")
                nc.vector.tensor_scalar(nmr[:], mv[:, 0:1], rstd[:], -1.0,
                                        OP.mult, OP.mult)
                ht = ln_tmp.tile([128, C], bf16, tag="lnh")
                nc.scalar.activation(ht[:], xt[:], AF.Identity,
                                     bias=nmr[:], scale=rstd[:])
                for cq in range(2):
                    tp = ps_tr.tile([128, 512], bf16, tag="tr")
                    for k in range(4):
                        c = 4 * cq + k
                        nc.tensor.transpose(tp[:, 128 * k:128 * (k + 1)],
                                            ht[:, 128 * c:128 * (c + 1)],
                                            ident[:])
                    nc.vector.tensor_copy(
                        hTall[:, 2 * cq:2 * cq + 2, :, 128 * t:128 * (t + 1)],
                        tp[:].rearrange("p (a b c) -> p a b c", b=2, c=128))

        with tc.tile_pool(name="att", bufs=1) as att:
            attT_sb = [att.tile([128, T], bf16, name=f"attT{p}")
                       for p in range(8)]
            attTp = [att.tile([128, 2, T], fp8, name=f"attTp{cp}")
                     for cp in range(4)]

            with tc.tile_pool(name="wqk", bufs=4) as wqk:
                wq_sb, wk_sb = {}, {}

                def qkw_dma(p8):
                    wq_sb[p8] = wqk.tile([128, 4, 2, 128], fp8, tag="wq",
                                         name=f"wq{p8}")
                    nc.sync.dma_start(
                        wq_sb[p8][:].rearrange("p a b c -> p (a b c)"),
                        wqb[p8, :, :])
                    wk_sb[p8] = wqk.tile([128, 4, 2, 128], fp8, tag="wk",
                                         name=f"wk{p8}")
                    nc.sync.dma_start(
                        wk_sb[p8][:].rearrange("p a b c -> p (a b c)"),
                        wkb[p8, :, :])

                for p in range(3):
                    qkw_dma(p)

                # ===================== phase 1: LN1 =========================
                with tc.tile_pool(name="ps_tr1", bufs=2, space="PSUM") as ps_tr:
                    layer_norm_to_hT(ps_tr)

                with tc.tile_pool(name="wv", bufs=1) as wv, \
                     tc.tile_pool(name="qkpool", bufs=3) as qkpool, \
                     tc.tile_pool(name="vpool", bufs=1) as vpool, \
                     tc.tile_pool(name="exp_pool", bufs=3) as exp_pool, \
                     tc.tile_pool(name="r_pool", bufs=2) as r_pool, \
                     tc.tile_pool(name="ps_qkv", bufs=2, space="PSUM") as ps_qkv, \
                     tc.tile_pool(name="ps_st", bufs=2, space="PSUM") as ps_st, \
                     tc.tile_pool(name="ps_av", bufs=2, space="PSUM") as ps_av:

                    wv_sb = [wv.tile([128, 2, C], fp8, name=f"wv{cp}")
                             for cp in range(4)]
                    for cp in range(4):
                        nc.sync.dma_start(wv_sb[cp][:], wvb[cp, :, :, :])
                    v_sb = [vpool.tile([128, H, VW], bf16, name=f"v{t}")
                            for t in range(NT)]

                    qk = {}      # p8 -> (qT tile, kT tile)
                    exps = {}    # p8 -> {e: [ex tiles per j]}
                    rrs = {}     # p8 -> rr tile

                    def qk_unit(p8, which, off):
                        """8 MMs (one c-contraction) for q or k, half width."""
                        if which == "q":
                            wsb, bias_nz = wq_sb[p8], flags["qb_nz"]
                            bias_sb, evac_act = qb_sb, flags["qb_nz"]
                        else:
                            wsb, bias_nz = wk_sb[p8], flags["kb_nz"]
                            bias_sb, evac_act = kb_sb, flags["kb_nz"]
                        if off == 0:
                            dst = qkpool.tile([128, T], bf16, tag=which,
                                              name=f"{which}{p8}")
                            qk.setdefault(p8, {})[which] = dst
                        else:
                            dst = qk[p8][which]
                        ps = ps_qkv.tile([128, 512], f32, tag="qkv")
                        for cp in range(4):
                            nc.tensor.matmul(
                                ps[:],
                                lhsT=wsb[:, cp, :, :],
                                rhs=hTall[:, cp, :, off:off + 512],
                                start=(cp == 0), stop=(cp == 3),
                                perf_mode=DR)
                        if evac_act:
                            nc.scalar.activation(
                                dst[:, off:off + 512], ps[:], AF.Identity,
                                bias=(bias_sb[:, p8:p8 + 1]
                                      if bias_nz else 0.0))
                        else:
                            nc.vector.tensor_copy(dst[:, off:off + 512], ps[:])

                    def v_unit(t, half):
                        """8 MMs for v[t], half of the heads."""
                        vt = v_sb[t]
                        if half == 0:
                            nc.gpsimd.memset(vt[:], 1.0)
                        ps = ps_qkv.tile([128, 512], f32, tag="qkv")
                        for cp in range(4):
                            nc.tensor.matmul(
                                ps[:],
                                lhsT=hTall[:, cp, :, 128 * t:128 * (t + 1)],
                                rhs=wv_sb[cp][:, :,
                                              512 * half:512 * (half + 1)],
                                start=(cp == 0), stop=(cp == 3),
                                perf_mode=DR)
                        ps3 = ps[:].rearrange("p (h d) -> p h d", d=HS)
                        hsl = slice(8 * half, 8 * (half + 1))
                        if flags["vb_nz"]:
                            vb3 = vb_sb[:].rearrange("p (h w) -> p h w", w=VW)
                            nc.vector.tensor_tensor(
                                vt[:, hsl, 0:HS], ps3, vb3[:, hsl, 0:HS],
                                OP.add)
                        else:
                            nc.vector.tensor_copy(vt[:, hsl, 0:HS], ps3)

                    def st_unit(p8, j, off):
                        """S^T for one j-tile, both heads (disjoint PE row
                        groups), in 512-col chunks through double-buffered
                        2-bank PSUM tiles; one paired exp call per chunk."""
                        assert off == 0
                        qT, kT = qk[p8]["q"], qk[p8]["k"]
                        W = T - 128 * j
                        ext = exp_pool.tile([128, 2, W], bf16, tag=f"exp{j}",
                                            name=f"ex{p8}_{j}")
                        exps.setdefault(p8, {})[j] = ext
                        for o in range(0, W, 512):
                            w = min(512, W - o)
                            st = ps_st.tile([128, 2, 512], f32, tag="st")
                            for e in (0, 1):
                                po = 64 * e
                                nc.tensor.matmul(
                                    st[:, e, 0:w],
                                    lhsT=kT[po:po + 64, 128 * j:128 * (j + 1)],
                                    rhs=qT[po:po + 64,
                                           128 * j + o:128 * j + o + w],
                                    start=True, stop=True)
                            nc.scalar.activation(ext[:, :, o:o + w],
                                                 st[:, :, 0:w],
                                                 AF.Exp, scale=SM_SCALE)
                        for e in (0, 1):
                            # causal mask on the diagonal 128x128 block
                            # (GpSimd: keeps the ACT-lagged wait off DVE)
                            nc.gpsimd.tensor_tensor(ext[:, e, 0:128],
                                                    ext[:, e, 0:128],
                                                    mask_sb[:], OP.mult)

                    def attv_unit(p8, e, off):
                        """attV for head 2*p8+e over one output half in a
                        1-bank PSUM tile; evacuate per half; epilogue on the
                        pair's last half."""
                        h = 2 * p8 + e
                        po = 64 * e
                        avp = ps_av.tile([65, 512], f32, tag="av",
                                         name=f"av{h}_{off}")
                        js = [j for j in range(NT) if 128 * j < off + 512]
                        for j in js:
                            lo = max(off, 128 * j)
                            nc.tensor.matmul(
                                avp[0:65, lo - off:512],
                                lhsT=v_sb[j][:, h, 0:65],
                                rhs=exps[p8][j][:, e, lo - 128 * j:
                                                off + 512 - 128 * j],
                                start=(j == js[0]), stop=(j == js[-1]))
                        # spill unnormalized attT half + stage the sums row
                        nc.vector.tensor_copy(
                            attT_sb[p8][po:po + 64, off:off + 512],
                            avp[0:64, :])
                        if e == 0 and off == 0:
                            sums2 = r_pool.tile([128, T], f32, tag="sums",
                                                name=f"sums{p8}", bufs=1)
                            rrs[(p8, "sums")] = sums2
                        else:
                            sums2 = rrs[(p8, "sums")]
                        nc.vector.tensor_copy(
                            sums2[32 * e:32 * e + 1, off:off + 512],
                            avp[64:65, :])
                        if e == 1 and off == 512:
                            r2 = r_pool.tile([64, T], f32, tag="r2",
                                             name=f"r2_{p8}", bufs=2)
                            nc.vector.reciprocal_approx_fast(r2[:],
                                                             sums2[0:64, :])
                            # DMA round-trip broadcast: write rows 0/32 to
                            # DRAM, read back partition-replicated
                            nc.sync.dma_start(rscr[p8, :, :], r2[:, :])
                            rr = r_pool.tile([128, T], f32, tag="rr",
                                             name=f"rrp{p8}", bufs=2)
                            s0 = rscr[p8, 0, :]
                            s1 = rscr[p8, 32, :]
                            nc.sync.dma_start(
                                rr[0:64, :],
                                bass.AP(tensor=s0.tensor, offset=s0.offset,
                                        ap=[[0, 64], [1, T]]))
                            nc.sync.dma_start(
                                rr[64:128, :],
                                bass.AP(tensor=s1.tensor, offset=s1.offset,
                                        ap=[[0, 64], [1, T]]))
                            rrs[p8] = rr

                    def norm_unit(p8):
                        rrs.pop((p8, "sums"), None)
                        rr = rrs.pop(p8)
                        nc.vector.tensor_tensor(attTp[p8 // 2][:, p8 % 2, :],
                                                attT_sb[p8][:],
                                                rr[:], OP.mult)
                        del exps[p8]

                    # ============ phase 2+3: QKV + attention pipeline ========
                    # prologue: q/k for pair 0
                    for which in ("q", "k"):
                        for off in (0, 512):
                            qk_unit(0, which, off)
                    for p8 in range(10):
                        if p8 + 3 <= 7:
                            qkw_dma(p8 + 3)
                        st_units = [(j, 0) for j in range(NT)] \
                            if p8 < 8 else []
                        fill = []
                        if p8 < 7:
                            fill += [lambda w=w, off=off, p=p8 + 1:
                                     qk_unit(p, w, off)
                                     for w in ("q", "k") for off in (0, 512)]
                        if p8 == 0:
                            fill += [lambda t=t, hf=hf: v_unit(t, hf)
                                     for t in range(0, 4) for hf in (0, 1)]
                        elif p8 == 1:
                            fill += [lambda t=t, hf=hf: v_unit(t, hf)
                                     for t in range(4, 8) for hf in (0, 1)]
                        if p8 >= 2:
                            fill += [lambda e=e, off=off, p=p8 - 2:
                                     attv_unit(p, e, off)
                                     for e in (0, 1) for off in (0, 512)]
                        # interleave S^T j-units with filler matmul groups
                        nst, nfill = len(st_units), len(fill)
                        fi = 0
                        for si, (j, off) in enumerate(st_units):
                            st_unit(p8, j, off)
                            want = ((si + 1) * nfill) // nst
                            while fi < want:
                                fill[fi]()
                                fi += 1
                        while fi < nfill:
                            fill[fi]()
                            fi += 1
                        if p8 >= 3:
                            norm_unit(p8 - 3)
                    norm_unit(7)
            # wqk/wv/qk/exp pools closed here
            with tc.tile_pool(name="wp", bufs=1) as wp, \
                 tc.tile_pool(name="ps_proj", bufs=2, space="PSUM") as ps_proj:
                wp_sb = [wp.tile([128, 2, C], fp8, name=f"wp{cp}")
                         for cp in range(4)]
                for cp in range(4):
                    nc.sync.dma_start(
                        wp_sb[cp][:].rearrange("p a c -> p (a c)"),
                        wpb[cp, :, :, :].rearrange("p a c -> p (a c)"))

                # ========================= phase 4: proj ====================
                for t in range(NT):
                    ps = ps_proj.tile([128, C], f32, tag="mm")
                    for off in (0, 512):
                        for cp in range(4):
                            nc.tensor.matmul(
                                ps[:, off:off + 512],
                                lhsT=attTp[cp][:, :, 128 * t:128 * (t + 1)],
                                rhs=wp_sb[cp][:, :, off:off + 512],
                                start=(cp == 0), stop=(cp == 3),
                                perf_mode=DR)
                    # xnew = x + y (+ biasp)
                    nc.vector.tensor_tensor(x_sb[t][:], ps[:], x_sb[t][:],
                                            OP.add)
                    if flags["biasp_nz"]:
                        nc.vector.tensor_tensor(x_sb[t][:], x_sb[t][:],
                                                bp_sb[:], OP.add)
        # att pool closed here

        # ==================== phase 5: LN2 -> h2T ===========================
        with tc.tile_pool(name="ps_tr2", bufs=2, space="PSUM") as ps_tr:
            layer_norm_to_hT(ps_tr)

        # ========================== phase 6: FFN ============================
        with tc.tile_pool(name="f1pool", bufs=1) as f1pool, \
             tc.tile_pool(name="w2pool", bufs=1) as w2pool, \
             tc.tile_pool(name="f1stg", bufs=3) as f1stg:
            f1_sb = [f1pool.tile([128, 2, T], fp8, name=f"f1_{fp_}")
                     for fp_ in range(16)]
            w2_sb = [w2pool.tile([128, 2, C], fp8, name=f"w2_{fp_}")
                     for fp_ in range(16)]
            with tc.tile_pool(name="w1pool", bufs=4) as w1pool, \
                 tc.tile_pool(name="ps_ffn1", bufs=2, space="PSUM") as ps_ffn1:
                for f in range(NFF):
                    w1t = w1pool.tile([128, 4, 2, 128], fp8, tag="w1t")
                    nc.sync.dma_start(
                        w1t[:].rearrange("p a b c -> p (a b c)"), w1b[f, :, :])
                    ps = ps_ffn1.tile([128, T], f32, tag="mm")
                    for off in (0, 512):
                        for cp in range(4):
                            nc.tensor.matmul(
                                ps[:, off:off + 512],
                                lhsT=w1t[:, cp, :, :],
                                rhs=hTall[:, cp, :, off:off + 512],
                                start=(cp == 0), stop=(cp == 3),
                                perf_mode=DR)
                    f1s = f1stg.tile([128, T], bf16, tag="f1s")
                    nc.scalar.activation(f1s[:], ps[:], AF.Relu,
                                         bias=b1e_sb[:, f:f + 1])
                    nc.vector.tensor_copy(f1_sb[f // 2][:, f % 2, :], f1s[:])
                    if f % 2 == 0:
                        nc.sync.dma_start(
                            w2_sb[f // 2][:].rearrange("p a c -> p (a c)"),
                            w2b[f // 2, :, :, :].rearrange("p a c -> p (a c)"))

            # FFN2: 4-tile t-groups x both halves (8 PSUM banks); the first
            # group's output DMAs overlap the second group's matmuls
            with tc.tile_pool(name="ps_y2", bufs=1, space="PSUM") as ps_y2:
                for tg in (0, 1):
                    trange = range(4 * tg, 4 * tg + 4)
                    y2 = {(t, half): ps_y2.tile([128, 512], f32,
                                                tag=f"y2_{t % 4}_{half}",
                                                name=f"y2_{t}_{half}")
                          for t in trange for half in (0, 1)}
                    for fp_ in range(16):
                        for t in trange:
                            for half in (0, 1):
                                nc.tensor.matmul(
                                    y2[(t, half)][:],
                                    lhsT=f1_sb[fp_][:, :,
                                                    128 * t:128 * (t + 1)],
                                    rhs=w2_sb[fp_][:, :,
                                                   512 * half:512 * half + 512],
                                    start=(fp_ == 0), stop=(fp_ == 15),
                                    perf_mode=DR)
                    for t in trange:
                        for half in (0, 1):
                            off = 512 * half
                            nc.vector.tensor_tensor(
                                x_sb[t][:, off:off + 512],
                                y2[(t, half)][:],
                                x_sb[t][:, off:off + 512], OP.add)
                            if flags["bias2_nz"]:
                                nc.vector.tensor_tensor(
                                    x_sb[t][:, off:off + 512],
                                    x_sb[t][:, off:off + 512],
                                    b2_sb[:, off:off + 512], OP.add)
                        nc.sync.dma_start(outd[128 * t:128 * (t + 1), :],
                                          x_sb[t][:])
    nc.compile()
    return nc


def _get_nc(flags):
    key = tuple(sorted(flags.items()))
    if key not in _CACHE:
        _CACHE[key] = build(flags)
    return _CACHE[key]


# ----------------------------------------------------------------------------
# public entry point
# ----------------------------------------------------------------------------

def kernel(**inputs):
    from concourse import bass_utils
    x = np.asarray(inputs["x"], np.float32)
    d, flags = _prep(inputs)
    nc = _get_nc(flags)
    in_maps = []
    for b in range(B):
        m = dict(d)
        m["x"] = np.ascontiguousarray(x[b])
        in_maps.append(m)
    res = bass_utils.run_bass_kernel_spmd(nc, in_maps, core_ids=list(range(B)))
    out = np.stack([r["out"] for r in res.results]).astype(np.float32)
    return out


# revision 23
# speedup vs baseline: 1.1310x; 1.1310x over previous
"""Trainium2 Bass kernel for a dense transformer block (B=8, T=1024, C=1024, H=16, FF=4096).

Sharding: data-parallel over batch — one batch element per NeuronCore (8 cores),
no collectives. Host does weight fake-quantization (exact, per reference formula),
LayerNorm gamma/beta folding into the adjacent projections, transposition to the
matmul-friendly [K, N] layouts, and bf16 casting. The device kernel computes, per
core, the whole block for its batch element:

  h1T = LN1(x)^T           (bf16, C on partitions; PE transposes)
  per head-pair p (software pipeline):
    qT/kT [d, t] matmuls -> S^T[s,t] = kT.T qT (K=64, two heads in disjoint PE
    row groups) -> exp (ACT, scale=C^-0.5) -> causal mask mult on diagonal tiles
    -> attV for pair p-1: attT_aug[65, t] = v_aug.T @ expT (PSUM, row 64 =
    softmax sums) -> recip(sums row) -> DMA round-trip broadcast of r ->
    attT *= r
  y = attT.T @ WpT; xnew = x + y; h2T = LN2(xnew)^T
  f1T[ff, t] = relu(W1T.T @ h2T); y2 = f1T.T @ W2T; out = xnew + y2

All matmuls are bf16 with fp32 PSUM accumulation. The weight quantization grid
(multiples of 2^e with tiny integer multipliers) is exactly representable in bf16.
"""

import os
import numpy as np
import ml_dtypes

B, T, C, H = 8, 1024, 1024, 16
HS = C // H          # 64
FF = 4 * C           # 4096
EPS = 1e-5
NT = T // 128        # 8 t-tiles
NCI = C // 128       # 8 c-tiles
NFF = FF // 128      # 32 ff-tiles
VW = 66              # per-head stride in v_aug (64 v cols + ones col + pad)
SM_SCALE = 1.0 / 32.0  # C ** -0.5

_CACHE = {}


# ----------------------------------------------------------------------------
# host-side math (exact reference semantics)
# ----------------------------------------------------------------------------

def _quant_weight(W, e, b):
    W = np.asarray(W, np.float32)
    e = np.asarray(e, np.float32)
    b = np.asarray(b, np.float32)
    b_rel = np.maximum(b, 0.0)
    mn = np.where(b_rel > 0, -(2.0 ** (b_rel - 1)), 0.0)
    mx = np.where(b_rel > 0, 2.0 ** (b_rel - 1) - 1.0, 0.0)
    qw = np.clip((2.0 ** (-e)) * W, mn, mx)
    w = np.round(qw)  # round-half-even, same as jnp.round
    return ((2.0 ** e) * w).astype(np.float32)


def _prep(inputs):
    f32 = np.float32
    bf16 = ml_dtypes.bfloat16
    g1 = np.asarray(inputs["g1"], f32)
    be1 = np.asarray(inputs["be1"], f32)
    g2 = np.asarray(inputs["g2"], f32)
    be2 = np.asarray(inputs["be2"], f32)

    Wq = _quant_weight(inputs["Wq"], inputs["eq"], inputs["bq"])  # [H,HS,C]
    Wk = _quant_weight(inputs["Wk"], inputs["ek"], inputs["bk"])
    Wv = _quant_weight(inputs["Wv"], inputs["ev"], inputs["bv"])
    Wp = _quant_weight(inputs["Wp"], inputs["ep"], inputs["bp"])  # [C,C]
    W1 = _quant_weight(inputs["W1"], inputs["e1"], inputs["b1"])  # [FF,C]
    W2 = _quant_weight(inputs["W2"], inputs["e2"], inputs["b2"])  # [C,FF]

    def qkvT(W):
        # [H,HS,C] -> fold g1 -> [C, H*HS]
        Wf = W * g1[None, None, :]
        return np.ascontiguousarray(Wf.reshape(H * HS, C).T).astype(bf16)

    def qkv_bias(W):
        # [H,HS,C] @ be1 -> [H*HS] -> [128, 8] with (r, p) = bias[128p + r]
        bias = (W.reshape(H * HS, C) @ be1).astype(f32)
        return np.ascontiguousarray(bias.reshape(8, 128).T)

    fp8 = ml_dtypes.float8_e4m3fn

    def qkv_pair(W):
        # [C, H*HS] -> [head-pair, 128(c-row), c-pair, 2, 128] fp8 DoubleRow
        # stationary layout; one DMA per head pair.
        WT = np.asarray(qkvT(W), np.float32)  # [C, H*HS]
        return np.ascontiguousarray(
            WT.reshape(4, 2, 128, 8, 128)
            .transpose(3, 2, 0, 1, 4).reshape(8, 128, NCI * 128)).astype(fp8)

    def wv_pair(W):
        # [C, H*HS] -> [c-pair, 128(c-row), 2, H*HS] fp8 DoubleRow moving
        WT = np.asarray(qkvT(W), np.float32)
        return np.ascontiguousarray(
            WT.reshape(4, 2, 128, C).transpose(0, 2, 1, 3)).astype(fp8)

    d = {
        "wqb": qkv_pair(Wq), "wkb": qkv_pair(Wk), "wvb": wv_pair(Wv),
        "qb": qkv_bias(Wq), "kb": qkv_bias(Wk),
        "wpb": np.ascontiguousarray(
            Wp.T.reshape(4, 2, 128, C).transpose(0, 2, 1, 3)).astype(fp8),
        # W1T [C, FF] -> [f, c_row(128), c-pair, 2, f_col(128)] fp8
        "w1b": np.ascontiguousarray(
            (W1 * g2[None, :]).T.reshape(4, 2, 128, NFF, 128)
            .transpose(3, 2, 0, 1, 4).reshape(NFF, 128, NCI * 128)).astype(fp8),
        # W2T [FF, C] -> [f-pair, 128(f-row), 2, C] fp8 DoubleRow moving
        "w2b": np.ascontiguousarray(
            W2.T.reshape(16, 2, 128, C).transpose(0, 2, 1, 3)).astype(fp8),
    }
    # b1eff[ff] = bias1 + W1 @ be2 ; laid out [128, 32] (partition r, col f)
    b1eff = (np.asarray(inputs["bias1"], f32) + W1 @ be2).astype(f32)
    d["b1e"] = np.ascontiguousarray(b1eff.reshape(NFF, 128).T)
    # v bias (from be1 through Wv), padded into the VW-stride layout
    vb = (Wv.reshape(H * HS, C) @ be1).astype(f32)                       # [H*HS]
    vb_pad = np.zeros(H * VW, f32)
    for h in range(H):
        vb_pad[h * VW: h * VW + HS] = vb[h * HS:(h + 1) * HS]
    d["vbpad"] = vb_pad
    d["biasp"] = np.asarray(inputs["biasp"], f32)
    d["bias2"] = np.asarray(inputs["bias2"], f32)
    # causal mask for diagonal blocks in S^T orientation: keep t_local >= s_local
    mask = (np.arange(128)[None, :] >= np.arange(128)[:, None])
    d["mask"] = mask.astype(bf16)
    qb = qkv_bias(Wq)
    kb = qkv_bias(Wk)
    flags = {
        "vb_nz": bool(np.any(vb != 0)),
        "qb_nz": bool(np.any(qb != 0)),
        "kb_nz": bool(np.any(kb != 0)),
        "biasp_nz": bool(np.any(d["biasp"] != 0)),
        "bias2_nz": bool(np.any(d["bias2"] != 0)),
    }
    return d, flags


# ----------------------------------------------------------------------------
# device kernel
# ----------------------------------------------------------------------------

def build(flags):
    import concourse.bass as bass
    import concourse.tile as tile
    from concourse import bacc, mybir

    f32 = mybir.dt.float32
    bf16 = mybir.dt.bfloat16
    AF = mybir.ActivationFunctionType
    OP = mybir.AluOpType

    nc = bacc.Bacc("TRN2", target_bir_lowering=False)

    xd = nc.dram_tensor("x", [T, C], f32, kind="ExternalInput")
    fp8 = mybir.dt.float8e4
    DR = mybir.MatmulPerfMode.DoubleRow
    wqb = nc.dram_tensor("wqb", [8, 128, C], fp8, kind="ExternalInput")
    wkb = nc.dram_tensor("wkb", [8, 128, C], fp8, kind="ExternalInput")
    wvb = nc.dram_tensor("wvb", [4, 128, 2, C], fp8, kind="ExternalInput")
    qbd = nc.dram_tensor("qb", [128, 8], f32, kind="ExternalInput")
    kbd = nc.dram_tensor("kb", [128, 8], f32, kind="ExternalInput")
    wpb = nc.dram_tensor("wpb", [4, 128, 2, C], fp8, kind="ExternalInput")
    w1b = nc.dram_tensor("w1b", [NFF, 128, NCI * 128], fp8,
                         kind="ExternalInput")
    w2b = nc.dram_tensor("w2b", [16, 128, 2, C], fp8, kind="ExternalInput")
    b1ed = nc.dram_tensor("b1e", [128, NFF], f32, kind="ExternalInput")
    maskd = nc.dram_tensor("mask", [128, 128], bf16, kind="ExternalInput")
    vbpd = nc.dram_tensor("vbpad", [H * VW], f32, kind="ExternalInput")
    biaspd = nc.dram_tensor("biasp", [C], f32, kind="ExternalInput")
    bias2d = nc.dram_tensor("bias2", [C], f32, kind="ExternalInput")
    outd = nc.dram_tensor("out", [T, C], f32, kind="ExternalOutput")
    # softmax 1/sum rows round-trip scratch ("Internal" DRAM fails NEFF load
    # under axon/bass2jax, so expose as an ignored output)
    rscr = nc.dram_tensor("rscr", [8, 64, T], f32, kind="ExternalOutput")

    def bcast_dram_row(vec_ap, p, n):
        # DRAM [n] broadcast across p partitions -> AP [p, n]
        return bass.AP(tensor=vec_ap.tensor, offset=vec_ap.offset,
                       ap=[[0, p], [1, n]])

    with tile.TileContext(nc) as tc, \
         tc.tile_pool(name="consts", bufs=1) as consts, \
         tc.tile_pool(name="xpool", bufs=1) as xpool, \
         tc.tile_pool(name="hpool", bufs=1) as hpool, \
         tc.tile_pool(name="ln_tmp", bufs=3) as ln_tmp:

        # ---- constants ----
        from concourse.masks import make_identity
        ident = consts.tile([128, 128], bf16, name="ident")
        make_identity(nc, ident[:])
        qb_sb = consts.tile([128, 8], f32, name="qb_sb")
        kb_sb = consts.tile([128, 8], f32, name="kb_sb")
        b1e_sb = consts.tile([128, NFF], f32, name="b1e_sb")
        mask_sb = consts.tile([128, 128], bf16, name="mask_sb")
        eps_sb = consts.tile([128, 1], f32, name="eps_sb")
        nc.vector.memset(eps_sb[:], EPS)

        # ---- x tiles first (LN1 critical path), then the other consts ----
        x_sb = []
        for t in range(NT):
            xt = xpool.tile([128, C], f32, name=f"x{t}")
            nc.sync.dma_start(xt[:], xd[128 * t:128 * (t + 1), :])
            x_sb.append(xt)
        if flags["qb_nz"]:
            nc.sync.dma_start(qb_sb[:], qbd[:, :])
        if flags["kb_nz"]:
            nc.sync.dma_start(kb_sb[:], kbd[:, :])
        nc.sync.dma_start(b1e_sb[:], b1ed[:, :])
        nc.sync.dma_start(mask_sb[:], maskd[:, :])
        if flags["vb_nz"]:
            vb_sb = consts.tile([128, H * VW], f32, name="vb_sb")
            nc.sync.dma_start(vb_sb[:], bcast_dram_row(vbpd[:], 128, H * VW))
        if flags["biasp_nz"]:
            bp_sb = consts.tile([128, C], f32, name="bp_sb")
            nc.sync.dma_start(bp_sb[:], bcast_dram_row(biaspd[:], 128, C))
        if flags["bias2_nz"]:
            b2_sb = consts.tile([128, C], f32, name="b2_sb")
            nc.sync.dma_start(b2_sb[:], bcast_dram_row(bias2d[:], 128, C))

        hTp = [hpool.tile([128, 2, T], fp8, tag=f"hTp{cp}", name=f"hTp{cp}")
               for cp in range(4)]

        def layer_norm_to_hT(ps_tr):
            """LN over x tiles -> bf16 h tiles -> transpose into hT."""
            for t in range(NT):
                xt = x_sb[t]
                stats = ln_tmp.tile([128, 2, 6], f32, tag="lnstats")
                nc.vector.bn_stats(stats[:, 0, :], xt[:, 0:512])
                nc.vector.bn_stats(stats[:, 1, :], xt[:, 512:1024])
                mv = ln_tmp.tile([128, 2], f32, tag="lnmv")
                nc.vector.bn_aggr(mv[:], stats[:])
                rstd = ln_tmp.tile([128, 1], f32, tag="lnrstd")
                # rstd = 1 / sqrt(var + EPS)
                nc.scalar.activation(rstd[:], mv[:, 1:2], AF.Sqrt, bias=eps_sb[:])
                nc.vector.reciprocal(rstd[:], rstd[:])
                # nmr = -mean * rstd; h = x * rstd + nmr  (on ACT)
                nmr = ln_tmp.tile([128, 1], f32, tag="lnnmr")
                nc.vector.tensor_scalar(nmr[:], mv[:, 0:1], rstd[:], -1.0,
                                        OP.mult, OP.mult)
                ht = ln_tmp.tile([128, C], bf16, tag="lnh")
                nc.scalar.activation(ht[:], xt[:], AF.Identity,
                                     bias=nmr[:], scale=rstd[:])
                for c in range(NCI):
                    tp = ps_tr.tile([128, 128], bf16, tag="tr")
                    nc.tensor.transpose(tp[:], ht[:, 128 * c:128 * (c + 1)],
                                        ident[:])
                    nc.vector.tensor_copy(
                        hTp[c // 2][:, c % 2, 128 * t:128 * (t + 1)], tp[:])

        with tc.tile_pool(name="att", bufs=1) as att:
            attT_sb = [att.tile([128, T], bf16, name=f"attT{p}")
                       for p in range(8)]
            attTp = [att.tile([128, 2, T], fp8, name=f"attTp{cp}")
                     for cp in range(4)]

            with tc.tile_pool(name="wqk", bufs=4) as wqk:
                wq_sb, wk_sb = {}, {}

                def qkw_dma(p8):
                    wq_sb[p8] = wqk.tile([128, 4, 2, 128], fp8, tag="wq",
                                         name=f"wq{p8}")
                    nc.sync.dma_start(
                        wq_sb[p8][:].rearrange("p a b c -> p (a b c)"),
                        wqb[p8, :, :])
                    wk_sb[p8] = wqk.tile([128, 4, 2, 128], fp8, tag="wk",
                                         name=f"wk{p8}")
                    nc.sync.dma_start(
                        wk_sb[p8][:].rearrange("p a b c -> p (a b c)"),
                        wkb[p8, :, :])

                for p in range(3):
                    qkw_dma(p)

                # ===================== phase 1: LN1 =========================
                with tc.tile_pool(name="ps_tr1", bufs=2, space="PSUM") as ps_tr:
                    layer_norm_to_hT(ps_tr)

                with tc.tile_pool(name="wv", bufs=1) as wv, \
                     tc.tile_pool(name="qkpool", bufs=3) as qkpool, \
                     tc.tile_pool(name="vpool", bufs=1) as vpool, \
                     tc.tile_pool(name="exp_pool", bufs=3) as exp_pool, \
                     tc.tile_pool(name="r_pool", bufs=2) as r_pool, \
                     tc.tile_pool(name="ps_qkv", bufs=2, space="PSUM") as ps_qkv, \
                     tc.tile_pool(name="ps_st", bufs=2, space="PSUM") as ps_st, \
                     tc.tile_pool(name="ps_av", bufs=2, space="PSUM") as ps_av:

                    wv_sb = [wv.tile([128, 2, C], fp8, name=f"wv{cp}")
                             for cp in range(4)]
                    for cp in range(4):
                        nc.sync.dma_start(wv_sb[cp][:], wvb[cp, :, :, :])
                    v_sb = [vpool.tile([128, H, VW], bf16, name=f"v{t}")
                            for t in range(NT)]

                    qk = {}      # p8 -> (qT tile, kT tile)
                    exps = {}    # p8 -> {e: [ex tiles per j]}
                    rrs = {}     # p8 -> rr tile

                    def qk_unit(p8, which, off):
                        """8 MMs (one c-contraction) for q or k, half width."""
                        if which == "q":
                            wsb, bias_nz = wq_sb[p8], flags["qb_nz"]
                            bias_sb, evac_act = qb_sb, flags["qb_nz"]
                        else:
                            wsb, bias_nz = wk_sb[p8], flags["kb_nz"]
                            bias_sb, evac_act = kb_sb, flags["kb_nz"]
                        if off == 0:
                            dst = qkpool.tile([128, T], bf16, tag=which,
                                              name=f"{which}{p8}")
                            qk.setdefault(p8, {})[which] = dst
                        else:
                            dst = qk[p8][which]
                        ps = ps_qkv.tile([128, 512], f32, tag="qkv")
                        for cp in range(4):
                            nc.tensor.matmul(
                                ps[:],
                                lhsT=wsb[:, cp, :, :],
                                rhs=hTp[cp][:, :, off:off + 512],
                                start=(cp == 0), stop=(cp == 3),
                                perf_mode=DR)
                        if evac_act:
                            nc.scalar.activation(
                                dst[:, off:off + 512], ps[:], AF.Identity,
                                bias=(bias_sb[:, p8:p8 + 1]
                                      if bias_nz else 0.0))
                        else:
                            nc.vector.tensor_copy(dst[:, off:off + 512], ps[:])

                    def v_unit(t, half):
                        """8 MMs for v[t], half of the heads."""
                        vt = v_sb[t]
                        if half == 0:
                            nc.gpsimd.memset(vt[:], 1.0)
                        ps = ps_qkv.tile([128, 512], f32, tag="qkv")
                        for cp in range(4):
                            nc.tensor.matmul(
                                ps[:],
                                lhsT=hTp[cp][:, :, 128 * t:128 * (t + 1)],
                                rhs=wv_sb[cp][:, :,
                                              512 * half:512 * (half + 1)],
                                start=(cp == 0), stop=(cp == 3),
                                perf_mode=DR)
                        ps3 = ps[:].rearrange("p (h d) -> p h d", d=HS)
                        hsl = slice(8 * half, 8 * (half + 1))
                        if flags["vb_nz"]:
                            vb3 = vb_sb[:].rearrange("p (h w) -> p h w", w=VW)
                            nc.vector.tensor_tensor(
                                vt[:, hsl, 0:HS], ps3, vb3[:, hsl, 0:HS],
                                OP.add)
                        else:
                            nc.vector.tensor_copy(vt[:, hsl, 0:HS], ps3)

                    def st_unit(p8, j, off):
                        """S^T for one j-tile, both heads (disjoint PE row
                        groups), in 512-col chunks through double-buffered
                        2-bank PSUM tiles; one paired exp call per chunk."""
                        assert off == 0
                        qT, kT = qk[p8]["q"], qk[p8]["k"]
                        W = T - 128 * j
                        ext = exp_pool.tile([128, 2, W], bf16, tag=f"exp{j}",
                                            name=f"ex{p8}_{j}")
                        exps.setdefault(p8, {})[j] = ext
                        for o in range(0, W, 512):
                            w = min(512, W - o)
                            st = ps_st.tile([128, 2, 512], f32, tag="st")
                            for e in (0, 1):
                                po = 64 * e
                                nc.tensor.matmul(
                                    st[:, e, 0:w],
                                    lhsT=kT[po:po + 64, 128 * j:128 * (j + 1)],
                                    rhs=qT[po:po + 64,
                                           128 * j + o:128 * j + o + w],
                                    start=True, stop=True)
                            nc.scalar.activation(ext[:, :, o:o + w],
                                                 st[:, :, 0:w],
                                                 AF.Exp, scale=SM_SCALE)
                        for e in (0, 1):
                            # causal mask on the diagonal 128x128 block
                            # (GpSimd: keeps the ACT-lagged wait off DVE)
                            nc.gpsimd.tensor_tensor(ext[:, e, 0:128],
                                                    ext[:, e, 0:128],
                                                    mask_sb[:], OP.mult)

                    def attv_unit(p8, e, off):
                        """attV for head 2*p8+e over one output half in a
                        1-bank PSUM tile; evacuate per half; epilogue on the
                        pair's last half."""
                        h = 2 * p8 + e
                        po = 64 * e
                        avp = ps_av.tile([65, 512], f32, tag="av",
                                         name=f"av{h}_{off}")
                        js = [j for j in range(NT) if 128 * j < off + 512]
                        for j in js:
                            lo = max(off, 128 * j)
                            nc.tensor.matmul(
                                avp[0:65, lo - off:512],
                                lhsT=v_sb[j][:, h, 0:65],
                                rhs=exps[p8][j][:, e, lo - 128 * j:
                                                off + 512 - 128 * j],
                                start=(j == js[0]), stop=(j == js[-1]))
                        # spill unnormalized attT half + stage the sums row
                        nc.vector.tensor_copy(
                            attT_sb[p8][po:po + 64, off:off + 512],
                            avp[0:64, :])
                        if e == 0 and off == 0:
                            sums2 = r_pool.tile([128, T], f32, tag="sums",
                                                name=f"sums{p8}", bufs=1)
                            rrs[(p8, "sums")] = sums2
                        else:
                            sums2 = rrs[(p8, "sums")]
                        nc.vector.tensor_copy(
                            sums2[32 * e:32 * e + 1, off:off + 512],
                            avp[64:65, :])
                        if e == 1 and off == 512:
                            r2 = r_pool.tile([64, T], f32, tag="r2",
                                             name=f"r2_{p8}", bufs=2)
                            nc.vector.reciprocal_approx_fast(r2[:],
                                                             sums2[0:64, :])
                            # DMA round-trip broadcast: write rows 0/32 to
                            # DRAM, read back partition-replicated
                            nc.sync.dma_start(rscr[p8, :, :], r2[:, :])
                            rr = r_pool.tile([128, T], f32, tag="rr",
                                             name=f"rrp{p8}", bufs=2)
                            s0 = rscr[p8, 0, :]
                            s1 = rscr[p8, 32, :]
                            nc.sync.dma_start(
                                rr[0:64, :],
                                bass.AP(tensor=s0.tensor, offset=s0.offset,
                                        ap=[[0, 64], [1, T]]))
                            nc.sync.dma_start(
                                rr[64:128, :],
                                bass.AP(tensor=s1.tensor, offset=s1.offset,
                                        ap=[[0, 64], [1, T]]))
                            rrs[p8] = rr

                    def norm_unit(p8):
                        rrs.pop((p8, "sums"), None)
                        rr = rrs.pop(p8)
                        nc.vector.tensor_tensor(attTp[p8 // 2][:, p8 % 2, :],
                                                attT_sb[p8][:],
                                                rr[:], OP.mult)
                        del exps[p8]

                    # ============ phase 2+3: QKV + attention pipeline ========
                    # prologue: q/k for pair 0
                    for which in ("q", "k"):
                        for off in (0, 512):
                            qk_unit(0, which, off)
                    for p8 in range(10):
                        if p8 + 3 <= 7:
                            qkw_dma(p8 + 3)
                        st_units = [(j, 0) for j in range(NT)] \
                            if p8 < 8 else []
                        fill = []
                        if p8 < 7:
                            fill += [lambda w=w, off=off, p=p8 + 1:
                                     qk_unit(p, w, off)
                                     for w in ("q", "k") for off in (0, 512)]
                        if p8 == 0:
                            fill += [lambda t=t, hf=hf: v_unit(t, hf)
                                     for t in range(0, 4) for hf in (0, 1)]
                        elif p8 == 1:
                            fill += [lambda t=t, hf=hf: v_unit(t, hf)
                                     for t in range(4, 8) for hf in (0, 1)]
                        if p8 >= 2:
                            fill += [lambda e=e, off=off, p=p8 - 2:
                                     attv_unit(p, e, off)
                                     for e in (0, 1) for off in (0, 512)]
                        # interleave S^T j-units with filler matmul groups
                        nst, nfill = len(st_units), len(fill)
                        fi = 0
                        for si, (j, off) in enumerate(st_units):
                            st_unit(p8, j, off)
                            want = ((si + 1) * nfill) // nst
                            while fi < want:
                                fill[fi]()
                                fi += 1
                        while fi < nfill:
                            fill[fi]()
                            fi += 1
                        if p8 >= 3:
                            norm_unit(p8 - 3)
                    norm_unit(7)
            # wqk/wv/qk/exp pools closed here
            with tc.tile_pool(name="wp", bufs=1) as wp, \
                 tc.tile_pool(name="ps_proj", bufs=2, space="PSUM") as ps_proj:
                wp_sb = [wp.tile([128, 2, C], fp8, name=f"wp{cp}")
                         for cp in range(4)]
                for cp in range(4):
                    nc.sync.dma_start(
                        wp_sb[cp][:].rearrange("p a c -> p (a c)"),
                        wpb[cp, :, :, :].rearrange("p a c -> p (a c)"))

                # ========================= phase 4: proj ====================
                for t in range(NT):
                    ps = ps_proj.tile([128, C], f32, tag="mm")
                    for off in (0, 512):
                        for cp in range(4):
                            nc.tensor.matmul(
                                ps[:, off:off + 512],
                                lhsT=attTp[cp][:, :, 128 * t:128 * (t + 1)],
                                rhs=wp_sb[cp][:, :, off:off + 512],
                                start=(cp == 0), stop=(cp == 3),
                                perf_mode=DR)
                    # xnew = x + y (+ biasp)
                    nc.vector.tensor_tensor(x_sb[t][:], ps[:], x_sb[t][:],
                                            OP.add)
                    if flags["biasp_nz"]:
                        nc.vector.tensor_tensor(x_sb[t][:], x_sb[t][:],
                                                bp_sb[:], OP.add)
        # att pool closed here

        # ==================== phase 5: LN2 -> h2T ===========================
        with tc.tile_pool(name="ps_tr2", bufs=2, space="PSUM") as ps_tr:
            layer_norm_to_hT(ps_tr)

        # ========================== phase 6: FFN ============================
        with tc.tile_pool(name="f1pool", bufs=1) as f1pool, \
             tc.tile_pool(name="w2pool", bufs=1) as w2pool, \
             tc.tile_pool(name="f1stg", bufs=3) as f1stg:
            f1_sb = [f1pool.tile([128, 2, T], fp8, name=f"f1_{fp_}")
                     for fp_ in range(16)]
            w2_sb = [w2pool.tile([128, 2, C], fp8, name=f"w2_{fp_}")
                     for fp_ in range(16)]
            with tc.tile_pool(name="w1pool", bufs=4) as w1pool, \
                 tc.tile_pool(name="ps_ffn1", bufs=2, space="PSUM") as ps_ffn1:
                for f in range(NFF):
                    w1t = w1pool.tile([128, 4, 2, 128], fp8, tag="w1t")
                    nc.sync.dma_start(
                        w1t[:].rearrange("p a b c -> p (a b c)"), w1b[f, :, :])
                    ps = ps_ffn1.tile([128, T], f32, tag="mm")
                    for off in (0, 512):
                        for cp in range(4):
                            nc.tensor.matmul(
                                ps[:, off:off + 512],
                                lhsT=w1t[:, cp, :, :],
                                rhs=hTp[cp][:, :, off:off + 512],
                                start=(cp == 0), stop=(cp == 3),
                                perf_mode=DR)
                    f1s = f1stg.tile([128, T], bf16, tag="f1s")
                    nc.scalar.activation(f1s[:], ps[:], AF.Relu,
                                         bias=b1e_sb[:, f:f + 1])
                    nc.vector.tensor_copy(f1_sb[f // 2][:, f % 2, :], f1s[:])
                    if f % 2 == 0:
                        nc.sync.dma_start(
                            w2_sb[f // 2][:].rearrange("p a c -> p (a c)"),
                            w2b[f // 2, :, :, :].rearrange("p a c -> p (a c)"))

            # FFN2: 4-tile t-groups x both halves (8 PSUM banks); the first
            # group's output DMAs overlap the second group's matmuls
            with tc.tile_pool(name="ps_y2", bufs=1, space="PSUM") as ps_y2:
                for tg in (0, 1):
                    trange = range(4 * tg, 4 * tg + 4)
                    y2 = {(t, half): ps_y2.tile([128, 512], f32,
                                                tag=f"y2_{t % 4}_{half}",
                                                name=f"y2_{t}_{half}")
                          for t in trange for half in (0, 1)}
                    for fp_ in range(16):
                        for t in trange:
                            for half in (0, 1):
                                nc.tensor.matmul(
                                    y2[(t, half)][:],
                                    lhsT=f1_sb[fp_][:, :,
                                                    128 * t:128 * (t + 1)],
                                    rhs=w2_sb[fp_][:, :,
                                                   512 * half:512 * half + 512],
                                    start=(fp_ == 0), stop=(fp_ == 15),
                                    perf_mode=DR)
                    for t in trange:
                        for half in (0, 1):
                            off = 512 * half
                            nc.vector.tensor_tensor(
                                x_sb[t][:, off:off + 512],
                                y2[(t, half)][:],
                                x_sb[t][:, off:off + 512], OP.add)
                            if flags["bias2_nz"]:
                                nc.vector.tensor_tensor(
                                    x_sb[t][:, off:off + 512],
                                    x_sb[t][:, off:off + 512],
                                    b2_sb[:, off:off + 512], OP.add)
                        nc.sync.dma_start(outd[128 * t:128 * (t + 1), :],
                                          x_sb[t][:])
    nc.compile()
    return nc


def _get_nc(flags):
    key = tuple(sorted(flags.items()))
    if key not in _CACHE:
        _CACHE[key] = build(flags)
    return _CACHE[key]


# ----------------------------------------------------------------------------
# public entry point
# ----------------------------------------------------------------------------

def kernel(**inputs):
    from concourse import bass_utils
    x = np.asarray(inputs["x"], np.float32)
    d, flags = _prep(inputs)
    nc = _get_nc(flags)
    in_maps = []
    for b in range(B):
        m = dict(d)
        m["x"] = np.ascontiguousarray(x[b])
        in_maps.append(m)
    res = bass_utils.run_bass_kernel_spmd(nc, in_maps, core_ids=list(range(B)))
    out = np.stack([r["out"] for r in res.results]).astype(np.float32)
    return out


# revision 24
# speedup vs baseline: 1.1400x; 1.0080x over previous
"""Trainium2 Bass kernel for a dense transformer block (B=8, T=1024, C=1024, H=16, FF=4096).

Sharding: data-parallel over batch — one batch element per NeuronCore (8 cores),
no collectives. Host does weight fake-quantization (exact, per reference formula),
LayerNorm gamma/beta folding into the adjacent projections, transposition to the
matmul-friendly [K, N] layouts, and bf16 casting. The device kernel computes, per
core, the whole block for its batch element:

  h1T = LN1(x)^T           (bf16, C on partitions; PE transposes)
  per head-pair p (software pipeline):
    qT/kT [d, t] matmuls -> S^T[s,t] = kT.T qT (K=64, two heads in disjoint PE
    row groups) -> exp (ACT, scale=C^-0.5) -> causal mask mult on diagonal tiles
    -> attV for pair p-1: attT_aug[65, t] = v_aug.T @ expT (PSUM, row 64 =
    softmax sums) -> recip(sums row) -> DMA round-trip broadcast of r ->
    attT *= r
  y = attT.T @ WpT; xnew = x + y; h2T = LN2(xnew)^T
  f1T[ff, t] = relu(W1T.T @ h2T); y2 = f1T.T @ W2T; out = xnew + y2

All matmuls are bf16 with fp32 PSUM accumulation. The weight quantization grid
(multiples of 2^e with tiny integer multipliers) is exactly representable in bf16.
"""

import os
import numpy as np
import ml_dtypes

B, T, C, H = 8, 1024, 1024, 16
HS = C // H          # 64
FF = 4 * C           # 4096
EPS = 1e-5
NT = T // 128        # 8 t-tiles
NCI = C // 128       # 8 c-tiles
NFF = FF // 128      # 32 ff-tiles
VW = 66              # per-head stride in v_aug (64 v cols + ones col + pad)
SM_SCALE = 1.0 / 32.0  # C ** -0.5

_CACHE = {}


# ----------------------------------------------------------------------------
# host-side math (exact reference semantics)
# ----------------------------------------------------------------------------

def _quant_weight(W, e, b):
    W = np.asarray(W, np.float32)
    e = np.asarray(e, np.float32)
    b = np.asarray(b, np.float32)
    b_rel = np.maximum(b, 0.0)
    mn = np.where(b_rel > 0, -(2.0 ** (b_rel - 1)), 0.0)
    mx = np.where(b_rel > 0, 2.0 ** (b_rel - 1) - 1.0, 0.0)
    qw = np.clip((2.0 ** (-e)) * W, mn, mx)
    w = np.round(qw)  # round-half-even, same as jnp.round
    return ((2.0 ** e) * w).astype(np.float32)


def _prep(inputs):
    f32 = np.float32
    bf16 = ml_dtypes.bfloat16
    g1 = np.asarray(inputs["g1"], f32)
    be1 = np.asarray(inputs["be1"], f32)
    g2 = np.asarray(inputs["g2"], f32)
    be2 = np.asarray(inputs["be2"], f32)

    Wq = _quant_weight(inputs["Wq"], inputs["eq"], inputs["bq"])  # [H,HS,C]
    Wk = _quant_weight(inputs["Wk"], inputs["ek"], inputs["bk"])
    Wv = _quant_weight(inputs["Wv"], inputs["ev"], inputs["bv"])
    Wp = _quant_weight(inputs["Wp"], inputs["ep"], inputs["bp"])  # [C,C]
    W1 = _quant_weight(inputs["W1"], inputs["e1"], inputs["b1"])  # [FF,C]
    W2 = _quant_weight(inputs["W2"], inputs["e2"], inputs["b2"])  # [C,FF]

    def qkvT(W):
        # [H,HS,C] -> fold g1 -> [C, H*HS]
        Wf = W * g1[None, None, :]
        return np.ascontiguousarray(Wf.reshape(H * HS, C).T).astype(bf16)

    def qkv_bias(W):
        # [H,HS,C] @ be1 -> [H*HS] -> [128, 8] with (r, p) = bias[128p + r]
        bias = (W.reshape(H * HS, C) @ be1).astype(f32)
        return np.ascontiguousarray(bias.reshape(8, 128).T)

    fp8 = ml_dtypes.float8_e4m3fn

    def qkv_pair(W):
        # [C, H*HS] -> [head-pair, 128(c-row), c-pair, 2, 128] fp8 DoubleRow
        # stationary layout; one DMA per head pair.
        WT = np.asarray(qkvT(W), np.float32)  # [C, H*HS]
        return np.ascontiguousarray(
            WT.reshape(4, 2, 128, 8, 128)
            .transpose(3, 2, 0, 1, 4).reshape(8, 128, NCI * 128)).astype(fp8)

    def wv_pair(W):
        # [C, H*HS] -> [c-pair, 128(c-row), 2, H*HS] fp8 DoubleRow moving
        WT = np.asarray(qkvT(W), np.float32)
        return np.ascontiguousarray(
            WT.reshape(4, 2, 128, C).transpose(0, 2, 1, 3)).astype(fp8)

    d = {
        "wqb": qkv_pair(Wq), "wkb": qkv_pair(Wk), "wvb": wv_pair(Wv),
        "qb": qkv_bias(Wq), "kb": qkv_bias(Wk),
        "wpb": np.ascontiguousarray(
            Wp.T.reshape(4, 2, 128, C).transpose(0, 2, 1, 3)).astype(fp8),
        # W1T [C, FF] -> [f, c_row(128), c-pair, 2, f_col(128)] fp8
        "w1b": np.ascontiguousarray(
            (W1 * g2[None, :]).T.reshape(4, 2, 128, NFF, 128)
            .transpose(3, 2, 0, 1, 4).reshape(NFF, 128, NCI * 128)).astype(fp8),
        # W2T [FF, C] -> [f-pair, 128(f-row), 2, C] fp8 DoubleRow moving
        "w2b": np.ascontiguousarray(
            W2.T.reshape(16, 2, 128, C).transpose(0, 2, 1, 3)).astype(fp8),
    }
    # b1eff[ff] = bias1 + W1 @ be2 ; laid out [128, 32] (partition r, col f)
    b1eff = (np.asarray(inputs["bias1"], f32) + W1 @ be2).astype(f32)
    d["b1e"] = np.ascontiguousarray(b1eff.reshape(NFF, 128).T)
    # v bias (from be1 through Wv), padded into the VW-stride layout
    vb = (Wv.reshape(H * HS, C) @ be1).astype(f32)                       # [H*HS]
    vb_pad = np.zeros(H * VW, f32)
    for h in range(H):
        vb_pad[h * VW: h * VW + HS] = vb[h * HS:(h + 1) * HS]
    d["vbpad"] = vb_pad
    d["biasp"] = np.asarray(inputs["biasp"], f32)
    d["bias2"] = np.asarray(inputs["bias2"], f32)
    # causal mask for diagonal blocks in S^T orientation: keep t_local >= s_local
    mask = (np.arange(128)[None, :] >= np.arange(128)[:, None])
    d["mask"] = mask.astype(bf16)
    qb = qkv_bias(Wq)
    kb = qkv_bias(Wk)
    flags = {
        "vb_nz": bool(np.any(vb != 0)),
        "qb_nz": bool(np.any(qb != 0)),
        "kb_nz": bool(np.any(kb != 0)),
        "biasp_nz": bool(np.any(d["biasp"] != 0)),
        "bias2_nz": bool(np.any(d["bias2"] != 0)),
    }
    return d, flags


# ----------------------------------------------------------------------------
# device kernel
# ----------------------------------------------------------------------------

def build(flags):
    import concourse.bass as bass
    import concourse.tile as tile
    from concourse import bacc, mybir

    f32 = mybir.dt.float32
    bf16 = mybir.dt.bfloat16
    AF = mybir.ActivationFunctionType
    OP = mybir.AluOpType

    nc = bacc.Bacc("TRN2", target_bir_lowering=False)

    xd = nc.dram_tensor("x", [T, C], f32, kind="ExternalInput")
    fp8 = mybir.dt.float8e4
    DR = mybir.MatmulPerfMode.DoubleRow
    wqb = nc.dram_tensor("wqb", [8, 128, C], fp8, kind="ExternalInput")
    wkb = nc.dram_tensor("wkb", [8, 128, C], fp8, kind="ExternalInput")
    wvb = nc.dram_tensor("wvb", [4, 128, 2, C], fp8, kind="ExternalInput")
    qbd = nc.dram_tensor("qb", [128, 8], f32, kind="ExternalInput")
    kbd = nc.dram_tensor("kb", [128, 8], f32, kind="ExternalInput")
    wpb = nc.dram_tensor("wpb", [4, 128, 2, C], fp8, kind="ExternalInput")
    w1b = nc.dram_tensor("w1b", [NFF, 128, NCI * 128], fp8,
                         kind="ExternalInput")
    w2b = nc.dram_tensor("w2b", [16, 128, 2, C], fp8, kind="ExternalInput")
    b1ed = nc.dram_tensor("b1e", [128, NFF], f32, kind="ExternalInput")
    maskd = nc.dram_tensor("mask", [128, 128], bf16, kind="ExternalInput")
    vbpd = nc.dram_tensor("vbpad", [H * VW], f32, kind="ExternalInput")
    biaspd = nc.dram_tensor("biasp", [C], f32, kind="ExternalInput")
    bias2d = nc.dram_tensor("bias2", [C], f32, kind="ExternalInput")
    outd = nc.dram_tensor("out", [T, C], f32, kind="ExternalOutput")
    # softmax 1/sum rows round-trip scratch ("Internal" DRAM fails NEFF load
    # under axon/bass2jax, so expose as an ignored output)
    rscr = nc.dram_tensor("rscr", [8, 64, T], f32, kind="ExternalOutput")

    def bcast_dram_row(vec_ap, p, n):
        # DRAM [n] broadcast across p partitions -> AP [p, n]
        return bass.AP(tensor=vec_ap.tensor, offset=vec_ap.offset,
                       ap=[[0, p], [1, n]])

    with tile.TileContext(nc) as tc, \
         tc.tile_pool(name="consts", bufs=1) as consts, \
         tc.tile_pool(name="xpool", bufs=1) as xpool, \
         tc.tile_pool(name="hpool", bufs=1) as hpool, \
         tc.tile_pool(name="ln_tmp", bufs=3) as ln_tmp:

        # ---- constants ----
        from concourse.masks import make_identity
        ident = consts.tile([128, 128], bf16, name="ident")
        make_identity(nc, ident[:])
        qb_sb = consts.tile([128, 8], f32, name="qb_sb")
        kb_sb = consts.tile([128, 8], f32, name="kb_sb")
        b1e_sb = consts.tile([128, NFF], f32, name="b1e_sb")
        mask_sb = consts.tile([128, 128], bf16, name="mask_sb")
        eps_sb = consts.tile([128, 1], f32, name="eps_sb")
        nc.vector.memset(eps_sb[:], EPS)

        # ---- x tiles first (LN1 critical path), then the other consts ----
        x_sb = []
        for t in range(NT):
            xt = xpool.tile([128, C], f32, name=f"x{t}")
            nc.sync.dma_start(xt[:], xd[128 * t:128 * (t + 1), :])
            x_sb.append(xt)
        if flags["qb_nz"]:
            nc.sync.dma_start(qb_sb[:], qbd[:, :])
        if flags["kb_nz"]:
            nc.sync.dma_start(kb_sb[:], kbd[:, :])
        nc.sync.dma_start(b1e_sb[:], b1ed[:, :])
        nc.sync.dma_start(mask_sb[:], maskd[:, :])
        if flags["vb_nz"]:
            vb_sb = consts.tile([128, H * VW], f32, name="vb_sb")
            nc.sync.dma_start(vb_sb[:], bcast_dram_row(vbpd[:], 128, H * VW))
        if flags["biasp_nz"]:
            bp_sb = consts.tile([128, C], f32, name="bp_sb")
            nc.sync.dma_start(bp_sb[:], bcast_dram_row(biaspd[:], 128, C))
        if flags["bias2_nz"]:
            b2_sb = consts.tile([128, C], f32, name="b2_sb")
            nc.sync.dma_start(b2_sb[:], bcast_dram_row(bias2d[:], 128, C))

        hTp = [hpool.tile([128, 2, T], fp8, tag=f"hTp{cp}", name=f"hTp{cp}")
               for cp in range(4)]

        def layer_norm_to_hT(ps_tr):
            """LN over x tiles -> bf16 h tiles -> transpose into hT."""
            for t in range(NT):
                xt = x_sb[t]
                stats = ln_tmp.tile([128, 2, 6], f32, tag="lnstats")
                nc.vector.bn_stats(stats[:, 0, :], xt[:, 0:512])
                nc.vector.bn_stats(stats[:, 1, :], xt[:, 512:1024])
                mv = ln_tmp.tile([128, 2], f32, tag="lnmv")
                nc.vector.bn_aggr(mv[:], stats[:])
                rstd = ln_tmp.tile([128, 1], f32, tag="lnrstd")
                # rstd = 1 / sqrt(var + EPS)
                nc.scalar.activation(rstd[:], mv[:, 1:2], AF.Sqrt, bias=eps_sb[:])
                nc.vector.reciprocal(rstd[:], rstd[:])
                # nmr = -mean * rstd; h = x * rstd + nmr  (on ACT)
                nmr = ln_tmp.tile([128, 1], f32, tag="lnnmr")
                nc.vector.tensor_scalar(nmr[:], mv[:, 0:1], rstd[:], -1.0,
                                        OP.mult, OP.mult)
                ht = ln_tmp.tile([128, C], bf16, tag="lnh")
                nc.scalar.activation(ht[:], xt[:], AF.Identity,
                                     bias=nmr[:], scale=rstd[:])
                for c in range(NCI):
                    tp = ps_tr.tile([128, 128], bf16, tag="tr")
                    nc.tensor.transpose(tp[:], ht[:, 128 * c:128 * (c + 1)],
                                        ident[:])
                    nc.vector.tensor_copy(
                        hTp[c // 2][:, c % 2, 128 * t:128 * (t + 1)], tp[:])

        with tc.tile_pool(name="att", bufs=1) as att:
            attT_sb = [att.tile([128, T], bf16, name=f"attT{p}")
                       for p in range(8)]
            attTp = [att.tile([128, 2, T], fp8, name=f"attTp{cp}")
                     for cp in range(4)]

            with tc.tile_pool(name="wqk", bufs=4) as wqk:
                wq_sb, wk_sb = {}, {}

                def qkw_dma(p8):
                    wq_sb[p8] = wqk.tile([128, 4, 2, 128], fp8, tag="wq",
                                         name=f"wq{p8}")
                    nc.sync.dma_start(
                        wq_sb[p8][:].rearrange("p a b c -> p (a b c)"),
                        wqb[p8, :, :])
                    wk_sb[p8] = wqk.tile([128, 4, 2, 128], fp8, tag="wk",
                                         name=f"wk{p8}")
                    nc.sync.dma_start(
                        wk_sb[p8][:].rearrange("p a b c -> p (a b c)"),
                        wkb[p8, :, :])

                for p in range(3):
                    qkw_dma(p)

                # ===================== phase 1: LN1 =========================
                with tc.tile_pool(name="ps_tr1", bufs=2, space="PSUM") as ps_tr:
                    layer_norm_to_hT(ps_tr)

                with tc.tile_pool(name="wv", bufs=1) as wv, \
                     tc.tile_pool(name="qkpool", bufs=3) as qkpool, \
                     tc.tile_pool(name="vpool", bufs=1) as vpool, \
                     tc.tile_pool(name="exp_pool", bufs=3) as exp_pool, \
                     tc.tile_pool(name="r_pool", bufs=2) as r_pool, \
                     tc.tile_pool(name="ps_qkv", bufs=2, space="PSUM") as ps_qkv, \
                     tc.tile_pool(name="ps_st", bufs=2, space="PSUM") as ps_st, \
                     tc.tile_pool(name="ps_av", bufs=2, space="PSUM") as ps_av:

                    wv_sb = [wv.tile([128, 2, C], fp8, name=f"wv{cp}")
                             for cp in range(4)]
                    for cp in range(4):
                        nc.sync.dma_start(wv_sb[cp][:], wvb[cp, :, :, :])
                    v_sb = [vpool.tile([128, H, VW], bf16, name=f"v{t}")
                            for t in range(NT)]

                    qk = {}      # p8 -> (qT tile, kT tile)
                    exps = {}    # p8 -> {e: [ex tiles per j]}
                    rrs = {}     # p8 -> rr tile

                    def qk_unit(p8, which, off):
                        """8 MMs (one c-contraction) for q or k, half width."""
                        if which == "q":
                            wsb, bias_nz = wq_sb[p8], flags["qb_nz"]
                            bias_sb, evac_act = qb_sb, flags["qb_nz"]
                        else:
                            wsb, bias_nz = wk_sb[p8], flags["kb_nz"]
                            bias_sb, evac_act = kb_sb, flags["kb_nz"]
                        if off == 0:
                            dst = qkpool.tile([128, T], bf16, tag=which,
                                              name=f"{which}{p8}")
                            qk.setdefault(p8, {})[which] = dst
                        else:
                            dst = qk[p8][which]
                        ps = ps_qkv.tile([128, 512], f32, tag="qkv")
                        for cp in range(4):
                            nc.tensor.matmul(
                                ps[:],
                                lhsT=wsb[:, cp, :, :],
                                rhs=hTp[cp][:, :, off:off + 512],
                                start=(cp == 0), stop=(cp == 3),
                                perf_mode=DR)
                        if evac_act:
                            nc.scalar.activation(
                                dst[:, off:off + 512], ps[:], AF.Identity,
                                bias=(bias_sb[:, p8:p8 + 1]
                                      if bias_nz else 0.0))
                        else:
                            nc.vector.tensor_copy(dst[:, off:off + 512], ps[:])

                    def v_unit(t, half):
                        """8 MMs for v[t], half of the heads."""
                        vt = v_sb[t]
                        if half == 0:
                            nc.gpsimd.memset(vt[:], 1.0)
                        ps = ps_qkv.tile([128, 512], f32, tag="qkv")
                        for cp in range(4):
                            nc.tensor.matmul(
                                ps[:],
                                lhsT=hTp[cp][:, :, 128 * t:128 * (t + 1)],
                                rhs=wv_sb[cp][:, :,
                                              512 * half:512 * (half + 1)],
                                start=(cp == 0), stop=(cp == 3),
                                perf_mode=DR)
                        ps3 = ps[:].rearrange("p (h d) -> p h d", d=HS)
                        hsl = slice(8 * half, 8 * (half + 1))
                        if flags["vb_nz"]:
                            vb3 = vb_sb[:].rearrange("p (h w) -> p h w", w=VW)
                            nc.vector.tensor_tensor(
                                vt[:, hsl, 0:HS], ps3, vb3[:, hsl, 0:HS],
                                OP.add)
                        else:
                            nc.vector.tensor_copy(vt[:, hsl, 0:HS], ps3)

                    def st_unit(p8, j, off):
                        """S^T for one j-tile, both heads (disjoint PE row
                        groups), in 512-col chunks through double-buffered
                        2-bank PSUM tiles; one paired exp call per chunk."""
                        assert off == 0
                        qT, kT = qk[p8]["q"], qk[p8]["k"]
                        W = T - 128 * j
                        ext = exp_pool.tile([128, 2, W], bf16, tag=f"exp{j}",
                                            name=f"ex{p8}_{j}")
                        exps.setdefault(p8, {})[j] = ext
                        for o in range(0, W, 512):
                            w = min(512, W - o)
                            st = ps_st.tile([128, 2, 512], f32, tag="st")
                            for e in (0, 1):
                                po = 64 * e
                                nc.tensor.matmul(
                                    st[:, e, 0:w],
                                    lhsT=kT[po:po + 64, 128 * j:128 * (j + 1)],
                                    rhs=qT[po:po + 64,
                                           128 * j + o:128 * j + o + w],
                                    start=True, stop=True)
                            nc.scalar.activation(ext[:, :, o:o + w],
                                                 st[:, :, 0:w],
                                                 AF.Exp, scale=SM_SCALE)
                        for e in (0, 1):
                            # causal mask on the diagonal 128x128 block
                            # (GpSimd: keeps the ACT-lagged wait off DVE)
                            nc.gpsimd.tensor_tensor(ext[:, e, 0:128],
                                                    ext[:, e, 0:128],
                                                    mask_sb[:], OP.mult)

                    def attv_unit(p8, e, off):
                        """attV for head 2*p8+e over one output half in a
                        1-bank PSUM tile; evacuate per half; epilogue on the
                        pair's last half."""
                        h = 2 * p8 + e
                        po = 64 * e
                        avp = ps_av.tile([65, 512], f32, tag="av",
                                         name=f"av{h}_{off}")
                        js = [j for j in range(NT) if 128 * j < off + 512]
                        for j in js:
                            lo = max(off, 128 * j)
                            nc.tensor.matmul(
                                avp[0:65, lo - off:512],
                                lhsT=v_sb[j][:, h, 0:65],
                                rhs=exps[p8][j][:, e, lo - 128 * j:
                                                off + 512 - 128 * j],
                                start=(j == js[0]), stop=(j == js[-1]))
                        # spill unnormalized attT half + stage the sums row
                        nc.vector.tensor_copy(
                            attT_sb[p8][po:po + 64, off:off + 512],
                            avp[0:64, :])
                        if e == 0 and off == 0:
                            sums2 = r_pool.tile([128, T], f32, tag="sums",
                                                name=f"sums{p8}", bufs=1)
                            rrs[(p8, "sums")] = sums2
                        else:
                            sums2 = rrs[(p8, "sums")]
                        nc.vector.tensor_copy(
                            sums2[32 * e:32 * e + 1, off:off + 512],
                            avp[64:65, :])
                        if e == 1 and off == 512:
                            r2 = r_pool.tile([64, T], f32, tag="r2",
                                             name=f"r2_{p8}", bufs=2)
                            nc.vector.reciprocal_approx_fast(r2[:],
                                                             sums2[0:64, :])
                            # DMA round-trip broadcast: write rows 0/32 to
                            # DRAM, read back partition-replicated
                            nc.sync.dma_start(rscr[p8, :, :], r2[:, :])
                            rr = r_pool.tile([128, T], f32, tag="rr",
                                             name=f"rrp{p8}", bufs=2)
                            s0 = rscr[p8, 0, :]
                            s1 = rscr[p8, 32, :]
                            nc.sync.dma_start(
                                rr[0:64, :],
                                bass.AP(tensor=s0.tensor, offset=s0.offset,
                                        ap=[[0, 64], [1, T]]))
                            nc.sync.dma_start(
                                rr[64:128, :],
                                bass.AP(tensor=s1.tensor, offset=s1.offset,
                                        ap=[[0, 64], [1, T]]))
                            rrs[p8] = rr

                    def norm_unit(p8):
                        rrs.pop((p8, "sums"), None)
                        rr = rrs.pop(p8)
                        nc.vector.tensor_tensor(attTp[p8 // 2][:, p8 % 2, :],
                                                attT_sb[p8][:],
                                                rr[:], OP.mult)
                        del exps[p8]

                    # ============ phase 2+3: QKV + attention pipeline ========
                    # prologue: q/k for pair 0
                    for which in ("q", "k"):
                        for off in (0, 512):
                            qk_unit(0, which, off)
                    for p8 in range(10):
                        if p8 + 3 <= 7:
                            qkw_dma(p8 + 3)
                        st_units = [(j, 0) for j in range(NT)] \
                            if p8 < 8 else []
                        fill = []
                        if p8 < 7:
                            fill += [lambda w=w, off=off, p=p8 + 1:
                                     qk_unit(p, w, off)
                                     for w in ("q", "k") for off in (0, 512)]
                        if p8 == 0:
                            fill += [lambda t=t, hf=hf: v_unit(t, hf)
                                     for t in range(0, 4) for hf in (0, 1)]
                        elif p8 == 1:
                            fill += [lambda t=t, hf=hf: v_unit(t, hf)
                                     for t in range(4, 8) for hf in (0, 1)]
                        if p8 >= 2:
                            fill += [lambda e=e, off=off, p=p8 - 2:
                                     attv_unit(p, e, off)
                                     for e in (0, 1) for off in (0, 512)]
                        # interleave S^T j-units with filler matmul groups
                        nst, nfill = len(st_units), len(fill)
                        fi = 0
                        for si, (j, off) in enumerate(st_units):
                            st_unit(p8, j, off)
                            want = ((si + 1) * nfill) // nst
                            while fi < want:
                                fill[fi]()
                                fi += 1
                        while fi < nfill:
                            fill[fi]()
                            fi += 1
                        if p8 >= 3:
                            norm_unit(p8 - 3)
                    norm_unit(7)
            # wqk/wv/qk/exp pools closed here
            with tc.tile_pool(name="wp", bufs=1) as wp, \
                 tc.tile_pool(name="ps_proj", bufs=2, space="PSUM") as ps_proj:
                wp_sb = [wp.tile([128, 2, C], fp8, name=f"wp{cp}")
                         for cp in range(4)]
                for cp in range(4):
                    nc.sync.dma_start(
                        wp_sb[cp][:].rearrange("p a c -> p (a c)"),
                        wpb[cp, :, :, :].rearrange("p a c -> p (a c)"))

                # ========================= phase 4: proj ====================
                # cp-outer so the cp<3 matmuls (not needing the last-normalized
                # head pairs) can run during the attention drain; only the
                # cp=3 accumulation waits on norm(6)/norm(7)
                for tg in (0, 1):
                    trange = range(4 * tg, 4 * tg + 4)
                    pst = {t: ps_proj.tile([128, C], f32, tag=f"pj{t % 4}",
                                           name=f"pj{t}", bufs=1)
                           for t in trange}
                    for cp in range(4):
                        for t in trange:
                            for off in (0, 512):
                                nc.tensor.matmul(
                                    pst[t][:, off:off + 512],
                                    lhsT=attTp[cp][:, :,
                                                   128 * t:128 * (t + 1)],
                                    rhs=wp_sb[cp][:, :, off:off + 512],
                                    start=(cp == 0), stop=(cp == 3),
                                    perf_mode=DR)
                    for t in trange:
                        # xnew = x + y (+ biasp)
                        nc.vector.tensor_tensor(x_sb[t][:], pst[t][:],
                                                x_sb[t][:], OP.add)
                        if flags["biasp_nz"]:
                            nc.vector.tensor_tensor(x_sb[t][:], x_sb[t][:],
                                                    bp_sb[:], OP.add)
        # att pool closed here

        # ==================== phase 5: LN2 -> h2T ===========================
        with tc.tile_pool(name="ps_tr2", bufs=2, space="PSUM") as ps_tr:
            layer_norm_to_hT(ps_tr)

        # ========================== phase 6: FFN ============================
        with tc.tile_pool(name="f1pool", bufs=1) as f1pool, \
             tc.tile_pool(name="w2pool", bufs=1) as w2pool, \
             tc.tile_pool(name="f1stg", bufs=3) as f1stg:
            f1_sb = [f1pool.tile([128, 2, T], fp8, name=f"f1_{fp_}")
                     for fp_ in range(16)]
            w2_sb = [w2pool.tile([128, 2, C], fp8, name=f"w2_{fp_}")
                     for fp_ in range(16)]
            with tc.tile_pool(name="w1pool", bufs=4) as w1pool, \
                 tc.tile_pool(name="ps_ffn1", bufs=2, space="PSUM") as ps_ffn1:
                for f in range(NFF):
                    w1t = w1pool.tile([128, 4, 2, 128], fp8, tag="w1t")
                    nc.sync.dma_start(
                        w1t[:].rearrange("p a b c -> p (a b c)"), w1b[f, :, :])
                    ps = ps_ffn1.tile([128, T], f32, tag="mm")
                    for off in (0, 512):
                        for cp in range(4):
                            nc.tensor.matmul(
                                ps[:, off:off + 512],
                                lhsT=w1t[:, cp, :, :],
                                rhs=hTp[cp][:, :, off:off + 512],
                                start=(cp == 0), stop=(cp == 3),
                                perf_mode=DR)
                    f1s = f1stg.tile([128, T], bf16, tag="f1s")
                    nc.scalar.activation(f1s[:], ps[:], AF.Relu,
                                         bias=b1e_sb[:, f:f + 1])
                    nc.vector.tensor_copy(f1_sb[f // 2][:, f % 2, :], f1s[:])
                    if f % 2 == 0:
                        nc.sync.dma_start(
                            w2_sb[f // 2][:].rearrange("p a c -> p (a c)"),
                            w2b[f // 2, :, :, :].rearrange("p a c -> p (a c)"))

            # FFN2: 4-tile t-groups x both halves (8 PSUM banks); the first
            # group's output DMAs overlap the second group's matmuls
            with tc.tile_pool(name="ps_y2", bufs=1, space="PSUM") as ps_y2:
                for tg in (0, 1):
                    trange = range(4 * tg, 4 * tg + 4)
                    y2 = {(t, half): ps_y2.tile([128, 512], f32,
                                                tag=f"y2_{t % 4}_{half}",
                                                name=f"y2_{t}_{half}")
                          for t in trange for half in (0, 1)}
                    for fp_ in range(16):
                        for t in trange:
                            for half in (0, 1):
                                nc.tensor.matmul(
                                    y2[(t, half)][:],
                                    lhsT=f1_sb[fp_][:, :,
                                                    128 * t:128 * (t + 1)],
                                    rhs=w2_sb[fp_][:, :,
                                                   512 * half:512 * half + 512],
                                    start=(fp_ == 0), stop=(fp_ == 15),
                                    perf_mode=DR)
                    for t in trange:
                        for half in (0, 1):
                            off = 512 * half
                            nc.vector.tensor_tensor(
                                x_sb[t][:, off:off + 512],
                                y2[(t, half)][:],
                                x_sb[t][:, off:off + 512], OP.add)
                            if flags["bias2_nz"]:
                                nc.vector.tensor_tensor(
                                    x_sb[t][:, off:off + 512],
                                    x_sb[t][:, off:off + 512],
                                    b2_sb[:, off:off + 512], OP.add)
                        nc.sync.dma_start(outd[128 * t:128 * (t + 1), :],
                                          x_sb[t][:])
    nc.compile()
    return nc


def _get_nc(flags):
    key = tuple(sorted(flags.items()))
    if key not in _CACHE:
        _CACHE[key] = build(flags)
    return _CACHE[key]


# ----------------------------------------------------------------------------
# public entry point
# ----------------------------------------------------------------------------

def kernel(**inputs):
    from concourse import bass_utils
    x = np.asarray(inputs["x"], np.float32)
    d, flags = _prep(inputs)
    nc = _get_nc(flags)
    in_maps = []
    for b in range(B):
        m = dict(d)
        m["x"] = np.ascontiguousarray(x[b])
        in_maps.append(m)
    res = bass_utils.run_bass_kernel_spmd(nc, in_maps, core_ids=list(range(B)))
    out = np.stack([r["out"] for r in res.results]).astype(np.float32)
    return out
